# revision 27
# baseline (speedup 1.0000x reference)
"""Trainium2 Bass kernel for top-1 MoE expert MLP (nn_Experts problem).

Strategy (expert-parallel, one expert per NeuronCore):
  - Routing is one-hot top-1: each token is processed by exactly one expert,
    so each core computes the MLP only for the tokens routed to its expert.
  - Host-side shard step: compute token->expert assignment from
    dispatch_tensor, gather each expert's tokens (transposed to [D, CAP]),
    and pack w1 into per-m1 column blocks so every DMA is contiguous and
    arrives in the order compute consumes it (HWDGE executes FIFO per
    issuing engine, so issue order == arrival order).
  - Device: h^T[F,CAP] = gelu(w1^T @ xT + b1); y[CAP,D] = (h @ w2) * gate.
  - Host-side unshard step: scatter per-expert rows back to token order and
    add the shared output bias b2.  No cross-core reduction is needed since
    token outputs are disjoint across experts.

MM_DT selects the tensor-engine dtype: "float32" is exact (4 cycles/row),
"float32r" streams at full rate (1 cycle/row, ~4x faster) but rounds matmul
operands to a 12-bit significand (~2e-4 relative).  For float32r the host
pre-rounds the operands (round-to-nearest-even at bit 12) so the BIR
verifier sees fp32r-clean inputs.
"""

import numpy as np

B, N, D, E, F = 8, 512, 1024, 8, 2048
T = B * N
P = 128
CAP = 576            # per-expert token capacity (max observed ~549 for T=4096, E=8)
NT = 5               # phase-B token tiles (last tile is 64 partitions)
LAST_M = CAP - 4 * P  # 64
KT1 = D // P         # 8  k-tiles for matmul1 (contract over D)
MT1 = F // P         # 16 m-tiles for matmul1 / k-tiles for matmul2
MM_DT = "float32r"   # matmul dtype: "float32" (exact) or "float32r" (fast)

# phase-A token chunking (fp32r needs moving dim >= 256 for full rate)
CHUNKS_A = ((0, 320), (320, 576))

_NC_CACHE = {}


def _round_fp32r(a):
    """Round-to-nearest-even at mantissa bit 12 (fp32r keeps the top 20 bits
    of an fp32 word: 1 sign + 8 exp + 11 explicit mantissa bits)."""
    u = a.view(np.uint32)
    lsb = (u >> 12) & 1
    u = u + 0x7FF + lsb
    u &= np.uint32(0xFFFFF000)
    return u.view(np.float32)


def _build_bass():
    import concourse.bacc as bacc
    import concourse.tile as tile
    from concourse import mybir

    f32 = mybir.dt.float32
    mm_dt = getattr(mybir.dt, MM_DT)

    nc = bacc.Bacc(None, target_bir_lowering=False)
    # head packs xT k-block 0 together with the w1 m1=0 column block so ONE
    # DMA unblocks the very first matmul.
    head = nc.declare_dram_parameter("head", [P, CAP + D], mm_dt,
                                     isOutput=False)
    gb = nc.declare_dram_parameter("gb", [P, MT1 + NT], f32, isOutput=False)
    xT = nc.declare_dram_parameter("xT", [D, CAP], mm_dt, isOutput=False)
    w1s = nc.declare_dram_parameter("w1s", [F, D], mm_dt, isOutput=False)
    w2s = nc.declare_dram_parameter("w2s", [F, D], mm_dt, isOutput=False)
    y = nc.declare_dram_parameter("y", [CAP, D], f32, isOutput=True)

    # w1 DMA block sizes in m1 units (m1=0 travels in `head`): fine-grained
    # at the front so early matmuls aren't gated, coarser later (fewer DMA
    # instructions -> fewer semaphore joins -> shorter Bacc event-sem
    # preamble/teardown).
    W1_BLOCKS = (1, 2, 2, 2, 2, 2, 2, 2)   # m1 = 1..15
    W2_BLK = 4  # k2-tiles per w2 DMA

    with tile.TileContext(nc) as tc:
        with (
            tc.tile_pool(name="gbp", bufs=1) as gbp,
            tc.tile_pool(name="hdp", bufs=1) as hdp,
            tc.tile_pool(name="xp", bufs=KT1 - 1) as xp,
            tc.tile_pool(name="w1p", bufs=5) as w1p,
            tc.tile_pool(name="w2p", bufs=MT1 // W2_BLK) as w2p,
            tc.tile_pool(name="hp", bufs=MT1) as hp,
            tc.tile_pool(name="stp", bufs=5) as stp,
            tc.tile_pool(name="psA", bufs=2, space="PSUM") as psA,
            tc.tile_pool(name="psB", bufs=4, space="PSUM") as psB,
        ):
            # DMA issue order == HWDGE arrival order (all on the sync-engine
            # ring): head (xT[0] + w1[m1=0]), rest of xT, gb, rest of w1
            # (phase A stream), then w2 (landing during phase A).
            HA = CAP + P  # head_a: xT[0] block + w1(m1=0, k=0) tile
            head_a = hdp.tile([P, HA], mm_dt, tag="ha")
            nc.sync.dma_start(out=head_a[:], in_=head[:, 0:HA])
            head_b = hdp.tile([P, CAP + D - HA], mm_dt, tag="hb")
            nc.sync.dma_start(out=head_b[:], in_=head[:, HA:])

            def load_x(k):
                t = xp.tile([P, CAP], mm_dt, tag="x", name=f"x_{k}")
                nc.sync.dma_start(out=t[:], in_=xT[k * P:(k + 1) * P, :])
                return t

            w1_blk, w1_off = [], []
            off = 1
            for nm in W1_BLOCKS:
                w1_off.append(off)
                off += nm

            def load_w1(j):
                nm = W1_BLOCKS[j]
                t = w1p.tile([P, nm, D], mm_dt, tag="w1", name=f"w1_{j}",
                             padded_shape=[P, 2, D])
                r0 = w1_off[j] * P
                src = w1s[r0:r0 + nm * P, :].rearrange(
                    "(j p) d -> p j d", p=P)
                nc.sync.dma_start(out=t[:], in_=src)
                return t

            x_sb = [head_a]
            for k in range(1, KT1):
                x_sb.append(load_x(k))
            gb_sb = gbp.tile([P, MT1 + NT], f32)
            nc.sync.dma_start(out=gb_sb[:], in_=gb[:, :])
            for j in range(len(W1_BLOCKS)):
                w1_blk.append(load_w1(j))
            w2_sb = []
            for j in range(MT1 // W2_BLK):
                t = w2p.tile([P, W2_BLK, D], mm_dt, tag="w2", name=f"w2_{j}")
                src = w2s[j * W2_BLK * P:(j + 1) * W2_BLK * P, :].rearrange(
                    "(j p) d -> p j d", p=P)
                nc.sync.dma_start(out=t[:], in_=src)
                w2_sb.append(t)

            def w1_lhs(m1, k):
                if m1 == 0:
                    if k == 0:
                        return head_a[:, CAP:CAP + P]
                    return head_b[:, (k - 1) * P:k * P]
                j = next(i for i in range(len(W1_BLOCKS))
                         if w1_off[i] <= m1 < w1_off[i] + W1_BLOCKS[i])
                return w1_blk[j][:, m1 - w1_off[j], k * P:(k + 1) * P]

            def primer(j, lhs1, rhs1):
                # borrows an (idle until phase B) psB slot; touches a fresh
                # w1 block on PE so later matmuls stay single-wait
                dummy = psB.tile([P, 512], f32, tag="psB", name=f"prime_{j}")
                nc.tensor.matmul(dummy[:2, :256], lhs1, rhs1,
                                 start=True, stop=True, skip_group_check=True)

            # Phase A: h^T[F, CAP] = gelu(w1^T @ x^T + b1)
            gelu = mybir.ActivationFunctionType.Gelu
            h_sb = []
            for m1 in range(MT1):
                if m1 > 0 and m1 in w1_off:
                    blk = w1_blk[w1_off.index(m1)]
                    primer(m1, blk[:, 0, 0:2], blk[:, 0, 0:256])
                pss = [psA.tile([P, b - a], f32, tag=f"psA{i}",
                                name=f"psA{i}_{m1}")
                       for i, (a, b) in enumerate(CHUNKS_A)]
                for k in range(KT1):
                    lhs = w1_lhs(m1, k)
                    st, sp = (k == 0), (k == KT1 - 1)
                    for i, (a, b) in enumerate(CHUNKS_A):
                        nc.tensor.matmul(pss[i][:], lhs, x_sb[k][:, a:b],
                                         start=st, stop=sp)
                h = hp.tile([P, CAP], mm_dt, tag="h", name=f"h_{m1}")
                bias = gb_sb[:, m1:m1 + 1]
                for i, (a, b) in enumerate(CHUNKS_A):
                    nc.scalar.activation(h[:, a:b], pss[i][:], gelu, bias=bias)
                h_sb.append(h)

            # Phase B: y[CAP, D] = (h @ w2) * gate  (64-row tile first)
            for m in [NT - 1] + list(range(NT - 1)):
                mm_ = P if m < NT - 1 else LAST_M
                pss = [psB.tile([P, 512], f32, tag="psB", name=f"psB_{m}_{n}")
                       for n in range(2)]
                for k2 in range(MT1):
                    lhs = h_sb[k2][:, m * P:m * P + mm_]
                    w2t = w2_sb[k2 // W2_BLK]
                    st, sp = (k2 == 0), (k2 == MT1 - 1)
                    for n in range(2):
                        nc.tensor.matmul(
                            pss[n][:mm_, :], lhs,
                            w2t[:, k2 % W2_BLK, n * 512:(n + 1) * 512],
                            start=st, stop=sp)
                gate = gb_sb[:, MT1 + m:MT1 + m + 1]
                stage = stp.tile([P, D], f32, tag="stage", name=f"stage_{m}")
                for n in range(2):
                    nc.vector.tensor_scalar_mul(
                        stage[:mm_, n * 512:(n + 1) * 512], pss[n][:mm_, :],
                        gate[:mm_, :])
                nc.sync.dma_start(out=y[m * P:m * P + mm_, :],
                                  in_=stage[:mm_, :])
    if not nc.is_finalized():
        nc.finalize()
    return nc


def _get_nc():
    if "nc" not in _NC_CACHE:
        _NC_CACHE["nc"] = _build_bass()
    return _NC_CACHE["nc"]


def kernel(x, dispatch_tensor, combine_tensor, w1, b1, w2, b2, **_):
    from concourse.bass_utils import run_bass_kernel_spmd

    x = np.ascontiguousarray(np.asarray(x, dtype=np.float32)).reshape(T, D)
    dispatch = np.asarray(dispatch_tensor, dtype=np.float32).reshape(T, E)
    combine = np.asarray(combine_tensor, dtype=np.float32).reshape(T, E)
    w1 = np.asarray(w1, dtype=np.float32)
    b1 = np.asarray(b1, dtype=np.float32)
    w2 = np.asarray(w2, dtype=np.float32)
    b2 = np.asarray(b2, dtype=np.float32)

    top = dispatch.argmax(-1)
    gate = combine.sum(-1)
    full = [np.nonzero(top == e)[0] for e in range(E)]
    idxs = [idx[:CAP] for idx in full]
    spill = [idx[CAP:] for idx in full]  # never non-empty for T=4096, E=8

    rnd = _round_fp32r if MM_DT == "float32r" else (lambda a: a)
    in_maps = []
    for e in range(E):
        idx = idxs[e]
        c = len(idx)
        xT = np.zeros((D, CAP), np.float32)
        xT[:, :c] = x[idx].T
        # w1s[m1*P+p, k*P+m] = w1[k*P+p, m1*P+m]: per-m1 [P, D] blocks whose
        # [:, k*P:(k+1)*P] slice is the lhsT k-tile for output tile m1.
        w1s = np.ascontiguousarray(
            w1[e].reshape(KT1, P, MT1, P).transpose(2, 1, 0, 3)
        ).reshape(F, D)
        gb = np.zeros((P, MT1 + NT), np.float32)
        gb[:, :MT1] = b1[e].reshape(MT1, P).T
        g = np.zeros(NT * P, np.float32)
        g[:c] = gate[idx]
        gb[:, MT1:] = g.reshape(NT, P).T
        xT, w1s = rnd(xT), rnd(w1s)
        in_maps.append({
            "head": np.ascontiguousarray(
                np.concatenate([xT[:P], w1s[:P]], axis=1)),
            "gb": gb,
            "xT": xT,
            "w1s": w1s,
            "w2s": rnd(np.ascontiguousarray(w2[e])),
        })

    global _LAST_IN_MAPS
    _LAST_IN_MAPS = in_maps
    nc = _get_nc()
    res = run_bass_kernel_spmd(nc, in_maps, list(range(E)))

    y_flat = np.empty((T, D), np.float32)
    for e in range(E):
        y_flat[idxs[e]] = res.results[e]["y"][:len(idxs[e])]
        if len(spill[e]):
            # capacity-overflow fallback (exact fp32 math on host); unused
            # for the reference shapes but keeps any input correct.
            import math

            erf = np.frompyfunc(math.erf, 1, 1)
            hs = x[spill[e]] @ w1[e] + b1[e]
            hs = hs * 0.5 * (1.0 + erf(hs / np.sqrt(2.0)).astype(np.float64))
            y_flat[spill[e]] = (hs @ w2[e]) * gate[spill[e]][:, None]
    return (y_flat + b2[None, :]).reshape(B, N, D)
